# revision 43
# baseline (speedup 1.0000x reference)
"""Deformable self-attention TRN2 kernel.

Sharding: 8 cores = batch(4) x head-group(2).  Each core handles one batch
element and 4 heads (128 of 256 v/out channels), producing a partial
out-projection; the host sums the two partials per batch and adds out_b.

Per-core pipeline (Tile-scheduled):
  1. x[b] -> SBUF, cast to bf16 zero-padded [128, 2cc, 98*98]
  2. conv3x3 (48 ch: px/py/aw) + base-coord matmul + v-proj, all bf16 on PE
  3. weight chain (DVE/ACT): floor/frac, DMA repack to slab layout, masks,
     bilinear factors, softmax(aw), corner weights (bf16 -> DRAM) and
     gather indices (int16, wrapped layout for dma_gather)
  4. per head: quad-token array (4 shifted copies + XBAR transpose);
     per (head, point, img-half): SBUF-source dma_gather (channel-major
     quads).  The Pool engine runs ONLY the 32 gathers (descriptor-gen
     bound, ~35us each); every other DMA rides HWDGE (sync/scalar).
  5. expand corner weights across 32-ch partition groups (one 3D-AP DMA
     per (h,p,half) on HWDGE), DVE multiply, PE corner-reduction (M=32
     matmul) -> sampled, out-proj.

Engine-op SBUF APs must start at partition 0/32/64/96 (HW quadrant rule);
DMAs may use arbitrary partition ranges - repacks/broadcasts ride on DMA.
"""

import numpy as np
import ml_dtypes

H = 96
W = 96
HW = H * W          # 9216
NH = 8
NP = 4
DIM = 256
HD = 32
NCORES = 8
NHL = 4             # heads per core
PAD = 98
NPIX_PAD = PAD * PAD
CH = 2304           # chain pixel quarter
NQ = 4
NC384 = 24
TOK = 9344          # token array cols (73*128)
TOKR = 73
VPADL = 97
VBUF = 9472
IDXMAX = 9312.0
HALF = HW // 2      # 4608
NCH = 12

_CACHE: dict = {}


def _build_nc():
    import concourse.bacc as bacc
    import concourse.bass as bass
    import concourse.mybir as mybir
    import concourse.tile as tile

    f32 = mybir.dt.float32
    bf16 = mybir.dt.bfloat16
    i16 = mybir.dt.int16

    nc = bacc.Bacc("TRN2", target_bir_lowering=False, debug=False,
                   enable_asserts=False, num_devices=NCORES)

    x_d = nc.dram_tensor("x", [DIM, HW], f32, kind="ExternalInput").ap()
    convw_d = nc.dram_tensor("convw", [128, 2, 9, 48], bf16,
                             kind="ExternalInput").ap()
    cbias_d = nc.dram_tensor("cbias", [48, 1], f32, kind="ExternalInput").ap()
    basew_d = nc.dram_tensor("basew", [2, 48], bf16, kind="ExternalInput").ap()
    basein_d = nc.dram_tensor("basein", [2, HW], bf16,
                              kind="ExternalInput").ap()
    vw_d = nc.dram_tensor("vw", [128, 2, 128], bf16, kind="ExternalInput").ap()
    vb_d = nc.dram_tensor("vb", [128, 1], f32, kind="ExternalInput").ap()
    w2_d = nc.dram_tensor("w2", [128, 2, 128], bf16, kind="ExternalInput").ap()
    redsel_d = nc.dram_tensor("redsel", [128, 32], bf16,
                              kind="ExternalInput").ap()
    sumsel_d = nc.dram_tensor("sumsel", [64, 16], f32,
                              kind="ExternalInput").ap()
    bcastsel_d = nc.dram_tensor("bcastsel", [16, 64], f32,
                                kind="ExternalInput").ap()
    ident_d = nc.dram_tensor("ident", [128, 32], bf16,
                             kind="ExternalInput").ap()
    out_d = nc.dram_tensor("out", [DIM, HW], f32, kind="ExternalOutput").ap()

    with tile.TileContext(nc) as tc:
        with tc.tile_pool(name="consts", bufs=1) as cpool:
            convw = cpool.tile([128, 2, 9, 48], bf16)
            nc.sync.dma_start(out=convw, in_=convw_d)
            cbias = cpool.tile([48, 1], f32)
            nc.sync.dma_start(out=cbias, in_=cbias_d)
            basew = cpool.tile([2, 48], bf16)
            nc.sync.dma_start(out=basew, in_=basew_d)
            vw = cpool.tile([128, 2, 128], bf16)
            nc.sync.dma_start(out=vw, in_=vw_d)
            vb = cpool.tile([128, 1], f32)
            nc.sync.dma_start(out=vb, in_=vb_d)
            w2 = cpool.tile([128, 2, 128], bf16)
            nc.sync.dma_start(out=w2, in_=w2_d)
            redsel = cpool.tile([128, 32], bf16)
            nc.sync.dma_start(out=redsel, in_=redsel_d)
            sumsel = cpool.tile([64, 16], f32)
            nc.sync.dma_start(out=sumsel, in_=sumsel_d)
            bcastsel = cpool.tile([16, 64], f32)
            nc.sync.dma_start(out=bcastsel, in_=bcastsel_d)
            ident = cpool.tile([128, 32], bf16)
            nc.sync.dma_start(out=ident, in_=ident_d)

            with tc.tile_pool(name="persist", bufs=1) as pp, \
                 tc.tile_pool(name="dpool", bufs=1, space="DRAM") as dp:
                v_cm = [pp.tile([64, VBUF], bf16, name=f"v_cm{i}")
                        for i in range(2)]
                for i in range(2):
                    nc.vector.memset(v_cm[i][:, 0:VPADL], 0.0)
                    nc.vector.memset(v_cm[i][:, VPADL + HW:VBUF], 0.0)
                W4flat = dp.tile([64, HW], bf16)
                idx_wrap = pp.tile([128, HW], i16)
                sampled = pp.tile([128, HW], bf16)

                with tc.tile_pool(name="pw", bufs=1) as pw:
                    # PXY packed: rows 32q..+16 = px(hp), +16..+32 = py(hp)
                    # AWp packed: rows 32q..+16 = aw(hp)
                    PXY = pw.tile([128, CH], f32)
                    AWp = pw.tile([128, CH], bf16)
                    _phase1(nc, tc, bass, mybir, x_d, basein_d, convw,
                            cbias, basew, vw, vb, v_cm, PXY, AWp)
                    _phase2_chain(nc, tc, bass, mybir, PXY, AWp, sumsel,
                                  bcastsel, W4flat, idx_wrap)
                _phase3_sample(nc, tc, bass, mybir, v_cm, W4flat,
                               idx_wrap, redsel, ident, sampled, w2, out_d)

    nc.compile()
    return nc


def _phase1(nc, tc, bass, mybir, x_d, basein_d, convw, cbias, basew, vw, vb,
            v_cm, PXY, AWp):
    f32 = mybir.dt.float32
    bf16 = mybir.dt.bfloat16
    Act = mybir.ActivationFunctionType

    with tc.tile_pool(name="xpad", bufs=1) as xp, \
         tc.tile_pool(name="ps1", bufs=3, space="PSUM") as ps1, \
         tc.tile_pool(name="ps2", bufs=3, space="PSUM") as ps2:
        # bf16 image, zero-padded; cast rides the SWDGE DMA (Pool is idle
        # here - the gathers come much later).  Only the 1-px border needs
        # zeroing.
        x_b = xp.tile([128, 2, NPIX_PAD], bf16)
        xb4 = x_b[:].rearrange("p cc (r c) -> p cc r c", c=PAD)
        nc.vector.memset(xb4[:, :, 0, :], 0.0)
        nc.vector.memset(xb4[:, :, 97, :], 0.0)
        nc.vector.memset(xb4[:, :, 1:97, 0:1], 0.0)
        nc.vector.memset(xb4[:, :, 1:97, 97:98], 0.0)
        for cc in range(2):
            dst = x_b[:, cc, :].rearrange("p (r c) -> p r c", c=PAD)
            xsrc = x_d[cc * 128:(cc + 1) * 128, :].rearrange(
                "p (r c) -> p r c", c=W)
            for rk in range(4):
                nc.gpsimd.dma_start(
                    out=dst[:, 1 + 24 * rk:1 + 24 * (rk + 1), 1:97],
                    in_=xsrc[:, 24 * rk:24 * (rk + 1), :])
        xv = [x_b[:, cc, :].rearrange("p (r c) -> p r c", c=PAD)
              for cc in range(2)]
        basein = xp.tile([2, HW], bf16)
        nc.sync.dma_start(out=basein, in_=basein_d)

        for j6o in range(6):
          for qo in range(4):
            n = 6 * qo + j6o
            ps = ps1.tile([48, 384], f32, tag="convps")
            first = True
            for cc in range(2):
                for t in range(9):
                    ky, kx = t // 3, t % 3
                    rhs = xv[cc][:, 4 * n + ky:4 * n + ky + 4, kx:kx + 96]
                    nc.tensor.matmul(
                        out=ps, lhsT=convw[:, cc, t, :], rhs=rhs,
                        start=first, stop=False)
                    first = False
            nc.tensor.matmul(out=ps, lhsT=basew,
                             rhs=basein[:, 384 * n:384 * (n + 1)],
                             start=False, stop=True)
            q, j6 = n // 6, n % 6
            sl = slice(384 * j6, 384 * (j6 + 1))
            nc.scalar.activation(out=PXY[32 * q:32 * q + 32, sl],
                                 in_=ps[0:32, :], func=Act.Identity,
                                 bias=cbias[0:32], scale=1.0)
            nc.scalar.activation(out=AWp[32 * q:32 * q + 16, sl],
                                 in_=ps[32:48, :], func=Act.Identity,
                                 bias=cbias[32:48], scale=1.0)

            psv = ps2.tile([128, 384], f32, tag="vps")
            for cc in range(2):
                rhsv = xv[cc][:, 4 * n + 1:4 * n + 5, 1:97]
                nc.tensor.matmul(out=psv, lhsT=vw[:, cc, :], rhs=rhsv,
                                 start=(cc == 0), stop=(cc == 1))
            nc.vector.tensor_scalar(
                out=v_cm[0][:, VPADL + 384 * n:VPADL + 384 * (n + 1)],
                in0=psv[0:64, :], scalar1=vb[0:64], scalar2=None,
                op0=mybir.AluOpType.add)
            nc.vector.tensor_scalar(
                out=v_cm[1][:, VPADL + 384 * n:VPADL + 384 * (n + 1)],
                in0=psv[64:128, :], scalar1=vb[64:128], scalar2=None,
                op0=mybir.AluOpType.add)


def _phase2_chain(nc, tc, bass, mybir, PXY, AWp, sumsel, bcastsel,
                  W4flat, idx_wrap):
    """Weight/index chain: (q,j6)-blocked slab repack + a spine sliced
    into six 384-col instances.

    With the conv emitted j6-major, slice j's inputs (all four quarters
    of column block j) are ready after conv position 4(j+1), so slices
    0..4 pipeline entirely under the conv and only slice 5 plus the idx
    packaging remains on the post-conv critical path.  Tiles are
    full-width and shared across slices via column views (Tile range
    tracking); scratch reuse is strictly same-tile/in-place so no
    cross-slice tag dependencies arise.
    """
    f32 = mybir.dt.float32
    bf16 = mybir.dt.bfloat16
    i16 = mybir.dt.int16
    i32 = mybir.dt.int32
    Alu = mybir.AluOpType
    Act = mybir.ActivationFunctionType

    with tc.tile_pool(name="chain", bufs=1) as chp, \
         tc.tile_pool(name="ps3", bufs=2, space="PSUM") as ps3, \
         tc.tile_pool(name="ps4", bufs=2, space="PSUM") as ps4:
        def t128(tag, dt=f32):
            return chp.tile([128, CH], dt, tag=tag, name=f"ch_{tag}")

        def t64(tag, dt=f32):
            return chp.tile([64, CH], dt, tag=tag, name=f"ch_{tag}")

        # (q, j6)-blocked repack: packed -> slab (row permutation only)
        PXYs = t128("cA")
        aws = t64("cH", bf16)
        for q in range(NQ):
            for j in range(6):
                cs = slice(384 * j, 384 * (j + 1))
                eng = nc.sync if (q + j) % 2 else nc.scalar
                eng.dma_start(out=PXYs[16 * q:16 * q + 16, cs],
                              in_=PXY[32 * q:32 * q + 16, cs])
                eng.dma_start(out=PXYs[64 + 16 * q:64 + 16 * q + 16, cs],
                              in_=PXY[32 * q + 16:32 * q + 32, cs])
                eng.dma_start(out=aws[16 * q:16 * q + 16, cs],
                              in_=AWp[32 * q:32 * q + 16, cs])

        # full-width tiles; slices operate on column views
        fls = t128("cB")       # int scratch, then floor (in place)
        frs = t128("cC")       # round, then fraction, then f1 (in place)
        dg = t128("cD")        # is_gt, clip scratch, omf, f0 (in place)
        m0 = t128("cE")
        m1 = t128("cF")
        exps = t64("cI")       # exp, later reused for t3 (same tile)
        awn = m1[0:64, :].bitcast(bf16)[:, 0:CH]  # rides m1's storage
        stg = t64("cK")        # fy0c / fy1c / flyc staging
        f0y = t64("cL")        # also t1
        f1y = t64("cM")        # also t2
        w4q = [chp.tile([64, CH], bf16, tag=f"cw{i}", name=f"w4q{i}")
               for i in range(4)]

        for j in range(6):
            sl = slice(384 * j, 384 * (j + 1))
            ints = fls[:, sl].bitcast(i32)
            nc.vector.tensor_copy(out=ints, in_=PXYs[:, sl])
            nc.vector.tensor_copy(out=frs[:, sl], in_=ints)
            nc.vector.tensor_tensor(out=dg[:, sl], in0=frs[:, sl],
                                    in1=PXYs[:, sl], op=Alu.is_gt)
            nc.vector.tensor_tensor(out=fls[:, sl], in0=frs[:, sl],
                                    in1=dg[:, sl], op=Alu.subtract)
            nc.vector.tensor_tensor(out=frs[:, sl], in0=PXYs[:, sl],
                                    in1=fls[:, sl], op=Alu.subtract)

            nc.vector.tensor_scalar(out=dg[:, sl], in0=fls[:, sl],
                                    scalar1=0.0, scalar2=95.0,
                                    op0=Alu.max, op1=Alu.min)
            nc.vector.tensor_tensor(out=m0[:, sl], in0=dg[:, sl],
                                    in1=fls[:, sl], op=Alu.is_equal)
            nc.vector.tensor_scalar(out=dg[:, sl], in0=fls[:, sl],
                                    scalar1=-1.0, scalar2=94.0,
                                    op0=Alu.max, op1=Alu.min)
            nc.vector.tensor_tensor(out=m1[:, sl], in0=dg[:, sl],
                                    in1=fls[:, sl], op=Alu.is_equal)
            nc.vector.tensor_scalar(out=dg[:, sl], in0=frs[:, sl],
                                    scalar1=-1.0, scalar2=1.0,
                                    op0=Alu.mult, op1=Alu.add)
            nc.vector.tensor_tensor(out=dg[:, sl], in0=dg[:, sl],
                                    in1=m0[:, sl], op=Alu.mult)   # f0
            nc.vector.tensor_tensor(out=frs[:, sl], in0=frs[:, sl],
                                    in1=m1[:, sl], op=Alu.mult)   # f1
            f0 = dg
            f1 = frs

            # softmax over points for this 384-block
            nc.scalar.activation(out=exps[:, sl], in_=aws[:, sl],
                                 func=Act.Exp, scale=1.0)
            pss = ps3.tile([16, 384], f32, tag="ssum")
            nc.tensor.matmul(out=pss, lhsT=sumsel, rhs=exps[:, sl],
                             start=True, stop=True)
            rsum = chp.tile([16, 384], f32, tag="cI2")
            nc.vector.reciprocal_approx_fast(out=rsum, in_=pss)
            psb = ps4.tile([64, 384], f32, tag="sbc")
            nc.tensor.matmul(out=psb, lhsT=bcastsel, rhs=rsum,
                             start=True, stop=True)
            nc.vector.tensor_tensor(out=awn[:, sl], in0=exps[:, sl],
                                    in1=psb, op=Alu.mult)

            # stage y-halves down to 0:64 and build corner weights
            nc.scalar.activation(out=stg[:, sl], in_=f0[64:128, sl],
                                 func=Act.Copy, scale=1.0)
            nc.vector.tensor_tensor(out=f0y[:, sl], in0=stg[:, sl],
                                    in1=awn[:, sl], op=Alu.mult)
            nc.scalar.activation(out=stg[:, sl], in_=f1[64:128, sl],
                                 func=Act.Copy, scale=1.0)
            nc.vector.tensor_tensor(out=f1y[:, sl], in0=stg[:, sl],
                                    in1=awn[:, sl], op=Alu.mult)
            nc.vector.tensor_tensor(out=w4q[0][:, sl], in0=f0[0:64, sl],
                                    in1=f0y[:, sl], op=Alu.mult)
            nc.vector.tensor_tensor(out=w4q[1][:, sl], in0=f1[0:64, sl],
                                    in1=f0y[:, sl], op=Alu.mult)
            nc.vector.tensor_tensor(out=w4q[2][:, sl], in0=f0[0:64, sl],
                                    in1=f1y[:, sl], op=Alu.mult)
            nc.vector.tensor_tensor(out=w4q[3][:, sl], in0=f1[0:64, sl],
                                    in1=f1y[:, sl], op=Alu.mult)

            # gather index: t = clip(y0*96 + x0 + 97, 0, 9312)
            nc.scalar.activation(out=stg[:, sl], in_=fls[64:128, sl],
                                 func=Act.Copy, scale=1.0)
            nc.vector.tensor_scalar(out=f0y[:, sl], in0=stg[:, sl],
                                    scalar1=96.0, scalar2=97.0,
                                    op0=Alu.mult, op1=Alu.add)    # t1
            nc.vector.tensor_tensor(out=f1y[:, sl], in0=f0y[:, sl],
                                    in1=fls[0:64, sl], op=Alu.add)  # t2
            nc.vector.tensor_scalar(out=exps[:, sl], in0=f1y[:, sl],
                                    scalar1=0.0, scalar2=IDXMAX,
                                    op0=Alu.max, op1=Alu.min)     # t3

        t3 = exps
        for c in range(4):
            for q in range(NQ):
                (nc.sync if q % 2 else nc.scalar).dma_start(
                    out=W4flat[16 * c:16 * c + 16, CH * q:CH * (q + 1)],
                    in_=w4q[c][16 * q:16 * q + 16, :])

        # int16 cast with within-row wrap permute: out[144a + b] = in[a + 16b]
        # idx16 rides the staging tile's bytes [0:4608) (f32 cols >= 1152
        # are untouched, so no dependency on the slice-5 flyc staging)
        idx16 = stg[:].bitcast(i16)[:, 0:CH]
        t3a = t3[:]
        in_ap = bass.AP(tensor=t3a.tensor, offset=t3a.offset,
                        ap=[t3a.ap[0], [1, 16], [16, 144]])
        out_ap = bass.AP(tensor=idx16.tensor, offset=idx16.offset,
                         ap=[idx16.ap[0], [144, 16], [1, 144]])
        nc.scalar.activation(out=out_ap, in_=in_ap, func=Act.Copy, scale=1.0)

        engines = [nc.sync, nc.scalar]
        k = 0
        for hp in range(16):
            for q in range(NQ):
                row = idx16[16 * q + hp:16 * q + hp + 1, :]
                src = bass.AP(tensor=row.tensor, offset=row.offset,
                              ap=[row.ap[0], [144, 16], [1, 144]])
                engines[k % 2].dma_start(
                    out=idx_wrap[0:16,
                                 576 * hp + 144 * q:576 * hp + 144 * (q + 1)],
                    in_=src)
                k += 1
        # tree replication of the wrapped rows: 16 -> 32 -> 64 -> 128
        nc.sync.dma_start(out=idx_wrap[16:32, :], in_=idx_wrap[0:16, :])
        nc.scalar.dma_start(out=idx_wrap[32:64, :], in_=idx_wrap[0:32, :])
        nc.sync.dma_start(out=idx_wrap[64:128, :], in_=idx_wrap[0:64, :])


def _phase3_sample(nc, tc, bass, mybir, v_cm, W4flat, idx_wrap, redsel,
                   ident, sampled, w2, out_d):
    """Gather + weighted corner/point reduction + out-projection.

    Token arrays are built by PE transposes (4 per 128-px block, one per
    quad shift, each [32,128] slice lifted straight out of v_cm) into
    PSUM, drained by ACT/DVE into a double-buffered tokens tile.  This
    keeps the Pool engine exclusively on the 32 dma_gathers and lets
    head h+1's tokens materialize while head h is still gathering.
    """
    f32 = mybir.dt.float32
    bf16 = mybir.dt.bfloat16
    Alu = mybir.AluOpType
    Act = mybir.ActivationFunctionType

    with tc.tile_pool(name="tokp", bufs=2) as tp, \
         tc.tile_pool(name="gpool", bufs=1) as gp, \
         tc.tile_pool(name="wpool", bufs=1) as wp, \
         tc.tile_pool(name="opool", bufs=2) as op, \
         tc.tile_pool(name="psr", bufs=4, space="PSUM") as psr, \
         tc.tile_pool(name="ptr", bufs=2, space="PSUM") as ptr, \
         tc.tile_pool(name="pso", bufs=2, space="PSUM") as pso:

        def build_tokens(h):
            tok = tp.tile([128, TOKR, 128], bf16, tag="tokens",
                          name=f"tokens{h}")
            for g in range(19):
                blocks = range(4 * g, min(4 * g + 4, TOKR))
                pt = ptr.tile([128, 4, 128], bf16, tag="pt")
                vt = v_cm[h // 2]
                r0 = 32 * (h % 2)
                for bi, b in enumerate(blocks):
                    for c, dlt in enumerate((0, 1, 96, 97)):
                        nc.tensor.matmul(
                            out=pt[:, bi, 32 * c:32 * c + 32],
                            lhsT=vt[r0:r0 + 32,
                                    dlt + 128 * b:dlt + 128 * b + 128],
                            rhs=ident[r0:r0 + 32, :],
                            is_transpose=True,
                            start=True, stop=True)
                dst = tok[:, 4 * g:4 * g + len(blocks), :]
                src = pt[:, 0:len(blocks), :]
                if g % 2 == 0:
                    nc.scalar.activation(out=dst, in_=src, func=Act.Copy,
                                         scale=1.0)
                else:
                    nc.vector.tensor_copy(out=dst, in_=src)
            return tok

        toks = {0: build_tokens(0)}
        seq = 0
        for h in range(NHL):
            tokens = toks.pop(h)
            for half in range(2):
                gt = []
                wexp = []
                for p in range(NP):
                    hp = h * 4 + p
                    g_t = gp.tile([128, 1, HALF], bf16, tag=f"g{seq % 6}",
                                  name=f"gt{hp}_{half}")
                    w_t = wp.tile([128, HALF], bf16, tag=f"w{p % 2}",
                                  name=f"wexp{hp}_{half}")
                    gt.append(g_t)
                    wexp.append(w_t)
                    seq += 1
                    # one 3D-AP broadcast DMA: rows {16c+hp} x32 replication
                    row0 = W4flat[hp:hp + 1, HALF * half:HALF * (half + 1)]
                    src = bass.AP(tensor=row0.tensor, offset=row0.offset,
                                  ap=[[0, 1], [16 * HW, 4], [0, 32],
                                      [1, HALF]])
                    nc.scalar.dma_start(
                        out=w_t[:].rearrange("p (a b) -> p a b", a=1),
                        in_=src)
                    nc.gpsimd.dma_gather(
                        g_t[:], tokens[:],
                        idx_wrap[:, 576 * hp + 288 * half:
                                 576 * hp + 288 * (half + 1)],
                        HALF, HALF, 128, transpose=True,
                        sbuf_tokens_per_rank=128,
                        sbuf_free_dim_per_rank=256,
                        single_packet=False)
                for p in range(NP):
                    nc.vector.tensor_tensor(out=gt[p][:, 0, :],
                                            in0=gt[p][:, 0, :], in1=wexp[p],
                                            op=Alu.mult)
                for n in range(NCH):
                    ng = NCH * half + n
                    bank = psr.tile([32, 384], f32, tag="red")
                    for p in range(NP):
                        nc.tensor.matmul(
                            out=bank, lhsT=redsel,
                            rhs=gt[p][:, 0, 384 * n:384 * (n + 1)],
                            start=(p == 0), stop=(p == 3))
                    nc.scalar.activation(
                        out=sampled[32 * h:32 * h + 32,
                                    384 * ng:384 * (ng + 1)],
                        in_=bank, func=Act.Copy, scale=1.0)
                if half == 0 and h + 1 < NHL:
                    toks[h + 1] = build_tokens(h + 1)
                if h == NHL - 1:
                    # image-half n-blocks are complete once the last head
                    # finishes this half: emit their out-projection now so
                    # it runs under the remaining gathers
                    for n in range(NCH * half, NCH * (half + 1)):
                        sl = slice(384 * n, 384 * (n + 1))
                        for oh in range(2):
                            ob = pso.tile([128, 384], f32, tag="ob")
                            nc.tensor.matmul(out=ob, lhsT=w2[:, oh, :],
                                             rhs=sampled[:, sl],
                                             start=True, stop=True)
                            osb = op.tile([128, 384], f32, tag="osb")
                            if (n + oh) % 2 == 0:
                                nc.vector.tensor_copy(out=osb, in_=ob)
                            else:
                                nc.scalar.activation(out=osb, in_=ob,
                                                     func=Act.Copy,
                                                     scale=1.0)
                            (nc.sync if (n + oh) % 2 else nc.scalar).dma_start(
                                out=out_d[oh * 128:(oh + 1) * 128, sl],
                                in_=osb)


def _host_inputs(inputs):
    x = np.asarray(inputs["x"], dtype=np.float32)
    kv_w = np.asarray(inputs["kv_w"], dtype=np.float32)
    kv_b = np.asarray(inputs["kv_b"], dtype=np.float32)
    off_w = np.asarray(inputs["off_w"], dtype=np.float32)
    off_b = np.asarray(inputs["off_b"], dtype=np.float32)
    aw_w = np.asarray(inputs["aw_w"], dtype=np.float32)
    aw_b = np.asarray(inputs["aw_b"], dtype=np.float32)
    out_w = np.asarray(inputs["out_w"], dtype=np.float32)

    sx = (W - 1.0) / W
    sy = (H - 1.0) / H

    redsel = np.zeros((128, 32), np.float32)
    for c in range(4):
        redsel[32 * c + np.arange(32), np.arange(32)] = 1.0
    sumsel = np.zeros((64, 16), np.float32)
    bcastsel = np.zeros((16, 64), np.float32)
    for q in range(4):
        for hh in range(4):
            for p in range(4):
                sumsel[16 * q + 4 * hh + p, 4 * q + hh] = 1.0
                bcastsel[4 * q + hh, 16 * q + 4 * hh + p] = 1.0

    basein = np.zeros((2, HW), np.float32)
    basein[0] = np.arange(HW) % W
    basein[1] = np.arange(HW) // W
    basew = np.zeros((2, 48), np.float32)
    basew[0, 0:16] = 1.0
    basew[1, 16:32] = 1.0

    bf = ml_dtypes.bfloat16
    in_maps = []
    for core in range(NCORES):
        b, hg = core // 2, core % 2
        heads = list(range(4 * hg, 4 * hg + 4))

        convw = np.zeros((128, 2, 9, 48), np.float32)
        cbias = np.zeros((48, 1), np.float32)
        for j, gh in enumerate(heads):
            for p in range(NP):
                hp = j * 4 + p
                wx = off_w[gh * 8 + p * 2 + 0] * sx
                wy = off_w[gh * 8 + p * 2 + 1] * sy
                wa = aw_w[gh * 4 + p]
                for t in range(9):
                    ky, kx = t // 3, t % 3
                    for cc in range(2):
                        csl = slice(cc * 128, (cc + 1) * 128)
                        convw[:, cc, t, hp] = wx[csl, ky, kx]
                        convw[:, cc, t, 16 + hp] = wy[csl, ky, kx]
                        convw[:, cc, t, 32 + hp] = wa[csl, ky, kx]
                cbias[hp, 0] = off_b[gh * 8 + p * 2 + 0] * sx
                cbias[16 + hp, 0] = off_b[gh * 8 + p * 2 + 1] * sy
                cbias[32 + hp, 0] = aw_b[gh * 4 + p]

        vw = np.zeros((128, 2, 128), np.float32)
        vrows = kv_w[DIM + hg * 128:DIM + (hg + 1) * 128, :]
        for cc in range(2):
            vw[:, cc, :] = vrows[:, cc * 128:(cc + 1) * 128].T
        vb = kv_b[DIM + hg * 128:DIM + (hg + 1) * 128].reshape(128, 1)

        w2 = np.zeros((128, 2, 128), np.float32)
        for halfi in range(2):
            w2[:, halfi, :] = out_w[halfi * 128:(halfi + 1) * 128,
                                    hg * 128:(hg + 1) * 128].T

        in_maps.append({
            "x": np.ascontiguousarray(x[b]),
            "convw": convw.astype(bf),
            "cbias": cbias,
            "basew": basew.astype(bf),
            "basein": basein.astype(bf),
            "vw": vw.astype(bf),
            "vb": np.ascontiguousarray(vb),
            "w2": w2.astype(bf),
            "redsel": redsel.astype(bf),
            "sumsel": sumsel,
            "bcastsel": bcastsel,
            "ident": np.tile(np.eye(32, dtype=np.float32), (4, 1)).astype(bf),
        })
    return in_maps


def kernel(**inputs):
    from concourse import bass_utils

    if "nc" not in _CACHE:
        _CACHE["nc"] = _build_nc()
    nc = _CACHE["nc"]

    in_maps = _host_inputs(inputs)
    res = bass_utils.run_bass_kernel_spmd(nc, in_maps,
                                          core_ids=list(range(NCORES)))
    out_b = np.asarray(inputs["out_b"], dtype=np.float32)
    out = np.zeros((4, DIM, HW), np.float32)
    for b in range(4):
        out[b] = (res.results[2 * b]["out"] + res.results[2 * b + 1]["out"]
                  + out_b[:, None])
    return out
